# revision 6
# baseline (speedup 1.0000x reference)
"""MoE experts MLP (gate/up + SiLU + down) on 8 TRN2 NeuronCores.

Expert-parallel: core e computes expert e end-to-end (E=8 experts, 8 cores).

Math per expert (reference):
    gate = x @ Wg.T          # [C,H] @ [H,I] -> [C,I]
    up   = x @ Wu.T
    inter = silu(gate) * up
    out  = inter @ Wd.T      # [C,I] @ [I,H] -> [C,H]

On-device layout trick: gate/up are computed *transposed* ([i, c] in PSUM)
so that every matmul operand uses the natural "DRAM row -> SBUF partition"
DMA layout, given host-side transposes:
    xT  = x.T   [H, C]   (tiles [h=128, c])   -> rhs of phase 1
    wgT = Wg.T  [H, I]   (tiles [h=128, i])   -> lhsT of phase 1
    wuT = Wu.T  [H, I]
    wdT = Wd.T  [I, H]   (tiles [i=128, h])   -> rhs of phase 2
    interT      [I, C]   (SBUF resident)      -> lhsT of phase 2
Phase-2 output lands as [c, h] in PSUM, DMA'd straight to out [C, H].

Compute in bf16 (fp32 PSUM accumulation), output fp32.
"""

import numpy as np
import ml_dtypes

import concourse.bass as bass
import concourse.bacc as bacc
import concourse.mybir as mybir
import concourse.tile as tile
from concourse.bass_utils import run_bass_kernel_spmd

E, C, H, I = 8, 1024, 2048, 5632
P = 128


def build_nc(C=C, H=H, I=I, IB=256, CB=512, HB=512):
    """Build the single-core Bass program (SPMD across 8 cores).

    IB: i-block width for phase-1 weight DMA (multiple of 128)
    CB: c-block width = N of phase-1 matmuls (<=512)
    HB: h-block width = N of phase-2 matmuls (<=512)
    """
    assert H % P == 0 and I % P == 0 and C % P == 0
    assert I % IB == 0 and C % CB == 0 and H % HB == 0 and IB % P == 0
    H_T, I_T = H // P, I // P
    bf16, f32 = mybir.dt.bfloat16, mybir.dt.float32

    # Bacc (not raw Bass): its compile() pass legalizes multi-sem waits
    # (generate_event_semaphores / move_matmul_waits_to_ldweights) — walrus
    # codegen only accepts one wait slot per instruction.
    nc = bacc.Bacc("TRN2", target_bir_lowering=False)
    xT = nc.dram_tensor("xT", [H, C], bf16, kind="ExternalInput")
    wg = nc.dram_tensor("wg", [H, I], bf16, kind="ExternalInput")
    wu = nc.dram_tensor("wu", [H, I], bf16, kind="ExternalInput")
    wd = nc.dram_tensor("wd", [I, H], bf16, kind="ExternalInput")
    out = nc.dram_tensor("out", [C, H], f32, kind="ExternalOutput")

    xT_r = xT.rearrange("(ho p) c -> p ho c", p=P)   # [128, H_T, C]
    wg_r = wg.rearrange("(ho p) i -> p ho i", p=P)   # [128, H_T, I]
    wu_r = wu.rearrange("(ho p) i -> p ho i", p=P)
    wd_r = wd.rearrange("(io p) h -> p io h", p=P)   # [128, I_T, H]
    out_r = out.rearrange("(co p) h -> p co h", p=P)  # [128, C_T, H]

    with tile.TileContext(nc) as tc:
        with tc.tile_pool(name="persist", bufs=1) as persist:
            # interT[i, c] resident in SBUF: [128, I_T, C] bf16
            interT = persist.tile([P, I_T, C], bf16, tag="interT")

            # ---- Phase 1: interT[i,c] = silu(WgT.T @ xT) * (WuT.T @ xT) ----
            with (
                tc.tile_pool(name="xpool", bufs=1) as xpool,
                tc.tile_pool(name="wpool", bufs=2) as wpool,
                tc.tile_pool(name="ps1", bufs=2, space="PSUM") as ps1,
                tc.tile_pool(name="actpool", bufs=3) as actpool,
            ):
                x_sb = xpool.tile([P, H_T, C], bf16, tag="x")
                nc.sync.dma_start(x_sb[:], xT_r[:])
                for ib in range(I // IB):
                    wg_sb = wpool.tile([P, H_T, IB], bf16, tag="wg")
                    wu_sb = wpool.tile([P, H_T, IB], bf16, tag="wu")
                    nc.sync.dma_start(wg_sb[:], wg_r[:, :, ib * IB:(ib + 1) * IB])
                    nc.sync.dma_start(wu_sb[:], wu_r[:, :, ib * IB:(ib + 1) * IB])
                    for it in range(IB // P):
                        i_tile = ib * (IB // P) + it
                        for cb in range(C // CB):
                            g_ps = ps1.tile([P, CB], f32, tag="gps")
                            u_ps = ps1.tile([P, CB], f32, tag="ups")
                            for h in range(H_T):
                                nc.tensor.matmul(
                                    g_ps[:],
                                    wg_sb[:, h, it * P:(it + 1) * P],
                                    x_sb[:, h, cb * CB:(cb + 1) * CB],
                                    start=(h == 0), stop=(h == H_T - 1),
                                )
                            for h in range(H_T):
                                nc.tensor.matmul(
                                    u_ps[:],
                                    wu_sb[:, h, it * P:(it + 1) * P],
                                    x_sb[:, h, cb * CB:(cb + 1) * CB],
                                    start=(h == 0), stop=(h == H_T - 1),
                                )
                            # Both multiply inputs are produced on ACT so the
                            # DVE TensorTensor needs a single sem wait (one
                            # wait slot per instruction).
                            silu_sb = actpool.tile([P, CB], f32, tag="silu")
                            nc.scalar.activation(
                                silu_sb[:], g_ps[:],
                                mybir.ActivationFunctionType.Silu,
                            )
                            u_sb = actpool.tile([P, CB], f32, tag="ucp")
                            nc.scalar.copy(out=u_sb[:], in_=u_ps[:])
                            nc.vector.tensor_tensor(
                                interT[:, i_tile, cb * CB:(cb + 1) * CB],
                                silu_sb[:], u_sb[:], mybir.AluOpType.mult,
                            )

            # ---- Phase 2: out[c,h] = interT.T @ WdT ----
            with (
                tc.tile_pool(name="wdpool", bufs=2) as wdpool,
                tc.tile_pool(name="ps2", bufs=2, space="PSUM") as ps2,
                tc.tile_pool(name="opool", bufs=3) as opool,
            ):
                for hb in range(H // HB):
                    wd_sb = wdpool.tile([P, I_T, HB], bf16, tag="wd")
                    nc.sync.dma_start(wd_sb[:], wd_r[:, :, hb * HB:(hb + 1) * HB])
                    for ct in range(C // P):
                        o_ps = ps2.tile([P, HB], f32, tag="ops")
                        for it in range(I_T):
                            nc.tensor.matmul(
                                o_ps[:],
                                interT[:, it, ct * P:(ct + 1) * P],
                                wd_sb[:, it, :],
                                start=(it == 0), stop=(it == I_T - 1),
                            )
                        o_sb = opool.tile([P, HB], f32, tag="osb")
                        nc.vector.tensor_copy(out=o_sb[:], in_=o_ps[:])
                        nc.sync.dma_start(
                            out_r[:, ct, hb * HB:(hb + 1) * HB], o_sb[:]
                        )
    nc.finalize()
    return nc


def _prep_in_maps(inputs, gate_proj, up_proj, down_proj):
    bf16 = ml_dtypes.bfloat16
    in_maps = []
    for e in range(inputs.shape[0]):
        in_maps.append({
            "xT": np.ascontiguousarray(inputs[e].T).astype(bf16),
            "wg": np.ascontiguousarray(gate_proj[e].T).astype(bf16),
            "wu": np.ascontiguousarray(up_proj[e].T).astype(bf16),
            "wd": np.ascontiguousarray(down_proj[e].T).astype(bf16),
        })
    return in_maps


def run(inputs, gate_proj, up_proj, down_proj, trace=False, **spmd_kwargs):
    n_cores = inputs.shape[0]
    nc = build_nc(C=inputs.shape[1], H=inputs.shape[2], I=gate_proj.shape[1])
    in_maps = _prep_in_maps(inputs, gate_proj, up_proj, down_proj)
    res = run_bass_kernel_spmd(
        nc, in_maps, core_ids=list(range(n_cores)), trace=trace, **spmd_kwargs
    )
    out = np.stack([r["out"] for r in res.results], axis=0)
    return out, res


def kernel(inputs, gate_proj, up_proj, down_proj):
    out, _ = run(inputs, gate_proj, up_proj, down_proj, trace=False)
    return out


# revision 8
# speedup vs baseline: 12.7172x; 12.7172x over previous
"""MoE experts MLP (gate/up + SiLU + down) on 8 TRN2 NeuronCores.

Expert-parallel: core e computes expert e end-to-end (E=8 experts, 8 cores).

Math per expert (reference):
    gate = x @ Wg.T          # [C,H] @ [H,I] -> [C,I]
    up   = x @ Wu.T
    inter = silu(gate) * up
    out  = inter @ Wd.T      # [C,I] @ [I,H] -> [C,H]

On-device layout trick: gate/up are computed *transposed* ([i, c] in PSUM)
so that every matmul operand uses the natural "DRAM row -> SBUF partition"
DMA layout, given host-side transposes:
    xT  = x.T   [H, C]   (tiles [h=128, c])   -> rhs of phase 1
    wgT = Wg.T  [H, I]   (tiles [h=128, i])   -> lhsT of phase 1
    wuT = Wu.T  [H, I]
    wdT = Wd.T  [I, H]   (tiles [i=128, h])   -> rhs of phase 2
    interT      [I, C]   (SBUF resident)      -> lhsT of phase 2
Phase-2 output lands as [c, h] in PSUM, DMA'd straight to out [C, H].

Compute in bf16 (fp32 PSUM accumulation), output fp32.
"""

import numpy as np
import ml_dtypes

import concourse.bass as bass
import concourse.bacc as bacc
import concourse.mybir as mybir
import concourse.tile as tile
from concourse.bass_utils import run_bass_kernel_spmd

E, C, H, I = 8, 1024, 2048, 5632
P = 128


def build_nc(C=C, H=H, I=I, IB=256, CB=512, HB=512, reps=1):
    """Build the single-core Bass program (SPMD across 8 cores).

    IB: i-block width for phase-1 weight DMA (multiple of 128)
    CB: c-block width = N of phase-1 matmuls (<=512)
    HB: h-block width = N of phase-2 matmuls (<=512)
    reps: replicate the whole computation (for timing-by-slope only)
    """
    assert H % P == 0 and I % P == 0 and C % P == 0
    assert I % IB == 0 and C % CB == 0 and H % HB == 0 and IB % P == 0
    H_T, I_T = H // P, I // P
    bf16, f32 = mybir.dt.bfloat16, mybir.dt.float32

    # Bacc (not raw Bass): its compile() pass legalizes multi-sem waits
    # (generate_event_semaphores / move_matmul_waits_to_ldweights) — walrus
    # codegen only accepts one wait slot per instruction.
    nc = bacc.Bacc("TRN2", target_bir_lowering=False)
    xT = nc.dram_tensor("xT", [H, C], bf16, kind="ExternalInput")
    wg = nc.dram_tensor("wg", [H, I], bf16, kind="ExternalInput")
    wu = nc.dram_tensor("wu", [H, I], bf16, kind="ExternalInput")
    wd = nc.dram_tensor("wd", [I, H], bf16, kind="ExternalInput")
    out = nc.dram_tensor("out", [C, H], f32, kind="ExternalOutput")

    xT_r = xT.rearrange("(ho p) c -> p ho c", p=P)   # [128, H_T, C]
    wg_r = wg.rearrange("(ho p) i -> p ho i", p=P)   # [128, H_T, I]
    wu_r = wu.rearrange("(ho p) i -> p ho i", p=P)
    wd_r = wd.rearrange("(io p) h -> p io h", p=P)   # [128, I_T, H]
    out_r = out.rearrange("(co p) h -> p co h", p=P)  # [128, C_T, H]

    with tile.TileContext(nc) as tc:
        for _rep in range(reps):
            _build_body(nc, tc, C, H, I, IB, CB, HB, H_T, I_T,
                        xT_r, wg_r, wu_r, wd_r, out_r)
    nc.finalize()
    return nc


def _build_body(nc, tc, C, H, I, IB, CB, HB, H_T, I_T,
                xT_r, wg_r, wu_r, wd_r, out_r):
    bf16, f32 = mybir.dt.bfloat16, mybir.dt.float32
    if True:
        with tc.tile_pool(name="persist", bufs=1) as persist:
            # interT[i, c] resident in SBUF: [128, I_T, C] bf16
            interT = persist.tile([P, I_T, C], bf16, tag="interT")

            # ---- Phase 1: interT[i,c] = silu(WgT.T @ xT) * (WuT.T @ xT) ----
            with (
                tc.tile_pool(name="xpool", bufs=1) as xpool,
                tc.tile_pool(name="wpool", bufs=2) as wpool,
                tc.tile_pool(name="ps1", bufs=2, space="PSUM") as ps1,
                tc.tile_pool(name="actpool", bufs=3) as actpool,
            ):
                x_sb = xpool.tile([P, H_T, C], bf16, tag="x")
                nc.sync.dma_start(x_sb[:], xT_r[:])
                for ib in range(I // IB):
                    wg_sb = wpool.tile([P, H_T, IB], bf16, tag="wg")
                    wu_sb = wpool.tile([P, H_T, IB], bf16, tag="wu")
                    nc.sync.dma_start(wg_sb[:], wg_r[:, :, ib * IB:(ib + 1) * IB])
                    nc.sync.dma_start(wu_sb[:], wu_r[:, :, ib * IB:(ib + 1) * IB])
                    for it in range(IB // P):
                        i_tile = ib * (IB // P) + it
                        for cb in range(C // CB):
                            g_ps = ps1.tile([P, CB], f32, tag="gps")
                            u_ps = ps1.tile([P, CB], f32, tag="ups")
                            for h in range(H_T):
                                nc.tensor.matmul(
                                    g_ps[:],
                                    wg_sb[:, h, it * P:(it + 1) * P],
                                    x_sb[:, h, cb * CB:(cb + 1) * CB],
                                    start=(h == 0), stop=(h == H_T - 1),
                                )
                            for h in range(H_T):
                                nc.tensor.matmul(
                                    u_ps[:],
                                    wu_sb[:, h, it * P:(it + 1) * P],
                                    x_sb[:, h, cb * CB:(cb + 1) * CB],
                                    start=(h == 0), stop=(h == H_T - 1),
                                )
                            # Both multiply inputs are produced on ACT so the
                            # DVE TensorTensor needs a single sem wait (one
                            # wait slot per instruction).
                            silu_sb = actpool.tile([P, CB], f32, tag="silu")
                            nc.scalar.activation(
                                silu_sb[:], g_ps[:],
                                mybir.ActivationFunctionType.Silu,
                            )
                            u_sb = actpool.tile([P, CB], f32, tag="ucp")
                            nc.scalar.copy(out=u_sb[:], in_=u_ps[:])
                            nc.vector.tensor_tensor(
                                interT[:, i_tile, cb * CB:(cb + 1) * CB],
                                silu_sb[:], u_sb[:], mybir.AluOpType.mult,
                            )

            # ---- Phase 2: out[c,h] = interT.T @ WdT ----
            with (
                tc.tile_pool(name="wdpool", bufs=2) as wdpool,
                tc.tile_pool(name="ps2", bufs=2, space="PSUM") as ps2,
                tc.tile_pool(name="opool", bufs=3) as opool,
            ):
                for hb in range(H // HB):
                    wd_sb = wdpool.tile([P, I_T, HB], bf16, tag="wd")
                    nc.sync.dma_start(wd_sb[:], wd_r[:, :, hb * HB:(hb + 1) * HB])
                    for ct in range(C // P):
                        o_ps = ps2.tile([P, HB], f32, tag="ops")
                        for it in range(I_T):
                            nc.tensor.matmul(
                                o_ps[:],
                                interT[:, it, ct * P:(ct + 1) * P],
                                wd_sb[:, it, :],
                                start=(it == 0), stop=(it == I_T - 1),
                            )
                        o_sb = opool.tile([P, HB], f32, tag="osb")
                        nc.vector.tensor_copy(out=o_sb[:], in_=o_ps[:])
                        nc.sync.dma_start(
                            out_r[:, ct, hb * HB:(hb + 1) * HB], o_sb[:]
                        )


def _prep_in_maps(inputs, gate_proj, up_proj, down_proj):
    bf16 = ml_dtypes.bfloat16
    in_maps = []
    for e in range(inputs.shape[0]):
        in_maps.append({
            "xT": np.ascontiguousarray(inputs[e].T).astype(bf16),
            "wg": np.ascontiguousarray(gate_proj[e].T).astype(bf16),
            "wu": np.ascontiguousarray(up_proj[e].T).astype(bf16),
            "wd": np.ascontiguousarray(down_proj[e].T).astype(bf16),
        })
    return in_maps


def run(inputs, gate_proj, up_proj, down_proj, trace=False, **spmd_kwargs):
    n_cores = inputs.shape[0]
    nc = build_nc(C=inputs.shape[1], H=inputs.shape[2], I=gate_proj.shape[1])
    in_maps = _prep_in_maps(inputs, gate_proj, up_proj, down_proj)
    res = run_bass_kernel_spmd(
        nc, in_maps, core_ids=list(range(n_cores)), trace=trace, **spmd_kwargs
    )
    out = np.stack([r["out"] for r in res.results], axis=0)
    return out, res


def kernel(inputs, gate_proj, up_proj, down_proj):
    out, _ = run(inputs, gate_proj, up_proj, down_proj, trace=False)
    return out


# revision 13
# speedup vs baseline: 18.0716x; 1.4210x over previous
"""MoE experts MLP (gate/up + SiLU + down) on 8 TRN2 NeuronCores.

Expert-parallel: core e computes expert e end-to-end (E=8 experts, 8 cores).

Math per expert (reference):
    gate = x @ Wg.T          # [C,H] @ [H,I] -> [C,I]
    up   = x @ Wu.T
    inter = silu(gate) * up
    out  = inter @ Wd.T      # [C,I] @ [I,H] -> [C,H]

On-device layout trick: gate/up are computed *transposed* ([i, c] in PSUM)
so that every matmul operand uses the natural "DRAM row -> SBUF partition"
DMA layout, given host-side transposes:
    xT  = x.T   [H, C]   (tiles [h=128, c])   -> rhs of phase 1
    wgT = Wg.T  [H, I]   (tiles [h=128, i])   -> lhsT of phase 1
    wuT = Wu.T  [H, I]
    wdT = Wd.T  [I, H]   (tiles [i=128, h])   -> rhs of phase 2
    interT      [I, C]   (SBUF resident)      -> lhsT of phase 2
Phase-2 output lands as [c, h] in PSUM, DMA'd straight to out [C, H].

Compute in bf16 (fp32 PSUM accumulation), output fp32.
"""

import numpy as np
import ml_dtypes

import concourse.bass as bass
import concourse.bacc as bacc
import concourse.mybir as mybir
import concourse.tile as tile
from concourse.bass_utils import run_bass_kernel_spmd

E, C, H, I = 8, 1024, 2048, 5632
P = 128


def build_nc(C=C, H=H, I=I, IB=256, CB=512, HB=512, reps=1, phases=(1, 2)):
    """Build the single-core Bass program (SPMD across 8 cores).

    IB: i-block width for phase-1 weight DMA (multiple of 128)
    CB: c-block width = N of phase-1 matmuls (<=512)
    HB: h-block width = N of phase-2 matmuls (<=512)
    reps: replicate the whole computation (for timing-by-slope only)
    """
    assert H % P == 0 and I % P == 0 and C % P == 0
    assert I % IB == 0 and C % CB == 0 and H % HB == 0 and IB % P == 0
    H_T, I_T = H // P, I // P
    bf16, f32 = mybir.dt.bfloat16, mybir.dt.float32

    # Bacc (not raw Bass): its compile() pass legalizes multi-sem waits
    # (generate_event_semaphores / move_matmul_waits_to_ldweights) — walrus
    # codegen only accepts one wait slot per instruction.
    nc = bacc.Bacc("TRN2", target_bir_lowering=False)
    xT = nc.dram_tensor("xT", [H, C], bf16, kind="ExternalInput")
    wg = nc.dram_tensor("wg", [H, I], bf16, kind="ExternalInput")
    wu = nc.dram_tensor("wu", [H, I], bf16, kind="ExternalInput")
    wd = nc.dram_tensor("wd", [I, H], bf16, kind="ExternalInput")
    out = nc.dram_tensor("out", [C, H], f32, kind="ExternalOutput")

    xT_r = xT.rearrange("(ho p) c -> p ho c", p=P)   # [128, H_T, C]
    wg_r = wg.rearrange("(ho p) i -> p ho i", p=P)   # [128, H_T, I]
    wu_r = wu.rearrange("(ho p) i -> p ho i", p=P)
    wd_r = wd.rearrange("(io p) h -> p io h", p=P)   # [128, I_T, H]
    out_r = out.rearrange("(co p) h -> p co h", p=P)  # [128, C_T, H]

    with tile.TileContext(nc) as tc:
        for _rep in range(reps):
            _build_body(nc, tc, C, H, I, IB, CB, HB, H_T, I_T,
                        xT_r, wg_r, wu_r, wd_r, out_r, phases=phases)
    nc.finalize()
    return nc


def _build_body(nc, tc, C, H, I, IB, CB, HB, H_T, I_T,
                xT_r, wg_r, wu_r, wd_r, out_r, phases=(1, 2)):
    bf16, f32 = mybir.dt.bfloat16, mybir.dt.float32
    if True:
        with tc.tile_pool(name="persist", bufs=1) as persist:
            # interT[i, c] resident in SBUF: [128, I_T, C] bf16
            interT = persist.tile([P, I_T, C], bf16, tag="interT")

            # ---- Phase 1: interT[i,c] = silu(WgT.T @ xT) * (WuT.T @ xT) ----
            if 1 in phases:
                _phase1(nc, tc, C, H, I, IB, CB, H_T, xT_r, wg_r, wu_r, interT)
            else:
                nc.any.memset(interT[:], 0.5)  # phase-2-only timing builds
            if 2 in phases:
                _phase2(nc, tc, C, H, I, HB, I_T, wd_r, out_r, interT)


def _phase1(nc, tc, C, H, I, IB, CB, H_T, xT_r, wg_r, wu_r, interT):
    bf16, f32 = mybir.dt.bfloat16, mybir.dt.float32
    P = 128
    if True:
        if True:
            with (
                tc.tile_pool(name="xpool", bufs=1) as xpool,
                tc.tile_pool(name="wpool", bufs=2) as wpool,
                tc.tile_pool(name="ps1", bufs=2, space="PSUM") as ps1,
                tc.tile_pool(name="actpool", bufs=3) as actpool,
            ):
                x_sb = xpool.tile([P, H_T, C], bf16, tag="x")
                nc.sync.dma_start(x_sb[:], xT_r[:])
                for ib in range(I // IB):
                    wg_sb = wpool.tile([P, H_T, IB], bf16, tag="wg")
                    wu_sb = wpool.tile([P, H_T, IB], bf16, tag="wu")
                    nc.sync.dma_start(wg_sb[:], wg_r[:, :, ib * IB:(ib + 1) * IB])
                    nc.sync.dma_start(wu_sb[:], wu_r[:, :, ib * IB:(ib + 1) * IB])
                    for it in range(IB // P):
                        i_tile = ib * (IB // P) + it
                        for cb in range(C // CB):
                            g_ps = ps1.tile([P, CB], f32, tag="gps")
                            u_ps = ps1.tile([P, CB], f32, tag="ups")
                            for h in range(H_T):
                                nc.tensor.matmul(
                                    g_ps[:],
                                    wg_sb[:, h, it * P:(it + 1) * P],
                                    x_sb[:, h, cb * CB:(cb + 1) * CB],
                                    start=(h == 0), stop=(h == H_T - 1),
                                )
                            for h in range(H_T):
                                nc.tensor.matmul(
                                    u_ps[:],
                                    wu_sb[:, h, it * P:(it + 1) * P],
                                    x_sb[:, h, cb * CB:(cb + 1) * CB],
                                    start=(h == 0), stop=(h == H_T - 1),
                                )
                            # Both multiply inputs are produced on ACT so the
                            # DVE TensorTensor needs a single sem wait (one
                            # wait slot per instruction).
                            silu_sb = actpool.tile([P, CB], f32, tag="silu")
                            nc.scalar.activation(
                                silu_sb[:], g_ps[:],
                                mybir.ActivationFunctionType.Silu,
                            )
                            u_sb = actpool.tile([P, CB], f32, tag="ucp")
                            nc.scalar.copy(out=u_sb[:], in_=u_ps[:])
                            nc.vector.tensor_tensor(
                                interT[:, i_tile, cb * CB:(cb + 1) * CB],
                                silu_sb[:], u_sb[:], mybir.AluOpType.mult,
                            )

def _phase2(nc, tc, C, H, I, HB, I_T, wd_r, out_r, interT):
    bf16, f32 = mybir.dt.bfloat16, mybir.dt.float32
    P = 128
    if True:
        if True:
            # ---- Phase 2: out[c,h] = interT.T @ WdT ----
            with (
                tc.tile_pool(name="wdpool", bufs=2) as wdpool,
                tc.tile_pool(name="ps2", bufs=2, space="PSUM") as ps2,
                tc.tile_pool(name="opool", bufs=3) as opool,
            ):
                for hb in range(H // HB):
                    wd_sb = wdpool.tile([P, I_T, HB], bf16, tag="wd")
                    nc.sync.dma_start(wd_sb[:], wd_r[:, :, hb * HB:(hb + 1) * HB])
                    for ct in range(C // P):
                        o_ps = ps2.tile([P, HB], f32, tag="ops")
                        for it in range(I_T):
                            nc.tensor.matmul(
                                o_ps[:],
                                interT[:, it, ct * P:(ct + 1) * P],
                                wd_sb[:, it, :],
                                start=(it == 0), stop=(it == I_T - 1),
                            )
                        o_sb = opool.tile([P, HB], f32, tag="osb")
                        nc.vector.tensor_copy(out=o_sb[:], in_=o_ps[:])
                        nc.sync.dma_start(
                            out_r[:, ct, hb * HB:(hb + 1) * HB], o_sb[:]
                        )


def _prep_in_maps(inputs, gate_proj, up_proj, down_proj):
    bf16 = ml_dtypes.bfloat16
    in_maps = []
    for e in range(inputs.shape[0]):
        in_maps.append({
            "xT": np.ascontiguousarray(inputs[e].T).astype(bf16),
            "wg": np.ascontiguousarray(gate_proj[e].T).astype(bf16),
            "wu": np.ascontiguousarray(up_proj[e].T).astype(bf16),
            "wd": np.ascontiguousarray(down_proj[e].T).astype(bf16),
        })
    return in_maps


def run(inputs, gate_proj, up_proj, down_proj, trace=False, **spmd_kwargs):
    n_cores = inputs.shape[0]
    nc = build_nc(C=inputs.shape[1], H=inputs.shape[2], I=gate_proj.shape[1])
    in_maps = _prep_in_maps(inputs, gate_proj, up_proj, down_proj)
    res = run_bass_kernel_spmd(
        nc, in_maps, core_ids=list(range(n_cores)), trace=trace, **spmd_kwargs
    )
    out = np.stack([r["out"] for r in res.results], axis=0)
    return out, res


def kernel(inputs, gate_proj, up_proj, down_proj):
    out, _ = run(inputs, gate_proj, up_proj, down_proj, trace=False)
    return out
